# revision 11
# baseline (speedup 1.0000x reference)
"""Bass/Trainium2 kernel for nn_BMGAE (LightGCN-style 2-layer propagation on
three bipartite graphs), sharded across 8 NeuronCores.

v2 strategy (vs baseline):
  - val factorization: val_e = f(src)*f(dst) with f = 1/(sqrt(deg)+eps), so the
    gathered table is premultiplied T_l = f * cur_l and the per-edge val
    multiply disappears. l2norm is scale-invariant, so the 1/(l+2) and f(s)
    factors vanish from the normalize path; only the next-layer table build
    needs f^2/2 (a per-node epilogue op).
  - tables stored as pre-split hi/lo fp16 pairs [hi64|lo64] (256B/row, same
    bytes as fp32): gathered tiles feed the fp16 matmul directly with no
    per-edge casts. Layer-0 table split on host; layer-1 split per-node in the
    epilogue before the AllGather.
  - big gather pieces (up to NI_MAX indices per DMAGather) sorted by
    (batch-of-16-blocks, chunk, block, dst) amortize the ~1us fixed SWDGE cost;
    PSUM holds 16 block accumulators (4 banks) double-buffered.

kernel(**inputs) takes the FULL unsharded inputs and returns the FULL output.
"""
import numpy as np

import concourse.tile as tile
from concourse import bass, bacc, mybir
from concourse.bass_utils import run_bass_kernel_spmd

P = 128
N_CORES = 8
D = 64
CH = 2 * D            # gathered row: [hi(64) | lo(64)] fp16 = 256B
LO_SCALE = 2048.0
EPS_DEG = 1e-8
EPS_RSQ = 1e-24
B_PP = 16             # blocks per PSUM batch (16 * 512B = 4 banks)
GQ = 4                # SWDGE queues
CHUNK = 32768         # int16 dma_gather index range per table chunk
NI_MAX = 4096         # max indices per dma_gather instruction

GRAPHS = [
    ("ui", "users", "items", "ui_src", "ui_dst", "ui_val"),
    ("ub", "users", "bundles", "ub_src", "ub_dst", "ub_val"),
    ("bi", "bundles", "items", "bi_src", "bi_dst", "bi_val"),
]


def _ceil(a, b):
    return -(-a // b)


class GraphPlan:
    """Host-side plan for one graph: permutation, piece/tile schedule."""

    def __init__(self, name, n, src, dst):
        self.name = name
        self.n = n
        deg = np.bincount(src, minlength=n)
        # f = 1/(sqrt(deg)+eps), matching reference fp32 ops
        self.f = (1.0 / (np.sqrt(deg.astype(np.float32)) + np.float32(EPS_DEG))
                  ).astype(np.float32)
        order = np.argsort(-deg, kind="stable")
        rank = np.empty(n, dtype=np.int64)
        rank[order] = np.arange(n)
        self.core_of = (rank % N_CORES).astype(np.int64)
        j = rank // N_CORES
        self.n_slice_pad = _ceil(_ceil(n, N_CORES), P) * P
        self.blocks = self.n_slice_pad // P
        self.slot_of = (j % self.blocks) * P + j // self.blocks
        self.n_pad = self.n_slice_pad * N_CORES
        self.gid_of = self.core_of * self.n_slice_pad + self.slot_of
        self.nchunks = _ceil(self.n_pad, CHUNK)
        self.nbatches = _ceil(self.blocks, B_PP)

        dst_g = self.gid_of[dst]
        src_core = self.core_of[src]
        src_slot = self.slot_of[src]

        # counts per (core, batch, chunk, block) -> shared run lengths (SPMD)
        blk = src_slot // P
        bat = blk // B_PP
        chk = dst_g // CHUNK
        counts = np.zeros((N_CORES, self.nbatches, self.nchunks, self.blocks),
                          dtype=np.int64)
        np.add.at(counts, (src_core, bat, chk, blk), 1)
        run_len = _ceil(np.maximum(counts.max(axis=0), 0), P) * P  # [bat,chk,blk]
        # zero-length runs stay zero
        run_len[counts.max(axis=0) == 0] = 0
        self.run_len = run_len

        # run order: (bat, chk, blk) with blk restricted to its batch
        # build tile schedule
        self.total_slots = int(run_len.sum())
        self.total_tiles = self.total_slots // P
        run_tile_off = np.zeros_like(run_len)
        t = 0
        self.batch_first_tile = []
        self.batch_tiles = []
        # pieces: list per batch of (chunk, tile_off, ntiles, segments)
        # segment: (block_in_batch, tile_off_in_piece, ntiles, is_first_spill)
        self.pieces = [[] for _ in range(self.nbatches)]
        tile_block = []
        max_ni_t = NI_MAX // P
        for b in range(self.nbatches):
            bt0 = t
            blo, bhi = b * B_PP, min((b + 1) * B_PP, self.blocks)
            seen = set()  # blocks already spilled once in this batch

            def close_piece(c, p_t0, p_nt, p_segs):
                self.pieces[b].append((c, p_t0, p_nt, p_segs))

            for c in range(self.nchunks):
                p_t0 = t
                p_nt = 0
                p_segs = []
                for bb in range(blo, bhi):
                    L = run_len[b, c, bb]
                    if L == 0:
                        continue
                    run_tile_off[b, c, bb] = t
                    ltiles = L // P
                    done = 0
                    while done < ltiles:
                        take = min(ltiles - done, max_ni_t - p_nt)
                        p_segs.append((bb - blo, p_nt, take, bb not in seen))
                        seen.add(bb)
                        tile_block.extend([bb - blo] * take)
                        t += take
                        p_nt += take
                        done += take
                        if p_nt == max_ni_t:
                            close_piece(c, p_t0, p_nt, p_segs)
                            p_t0 = t
                            p_nt = 0
                            p_segs = []
                if p_nt > 0:
                    close_piece(c, p_t0, p_nt, p_segs)
            # every block in the batch must have been spilled at least once
            assert len(seen) == bhi - blo, (self.name, b, seen)
            self.batch_first_tile.append(bt0)
            self.batch_tiles.append(t - bt0)
        assert t == self.total_tiles
        self.run_tile_off = run_tile_off
        self.tile_block = np.array(tile_block, dtype=np.int64)
        self.max_batch_tiles = max(self.batch_tiles)

        # ---- per-core slot arrays ----
        self.idx16 = np.zeros((N_CORES, P, self.total_slots // 16), np.int16)
        self.srcrel = np.full((N_CORES, P, self.total_tiles), -1.0, np.float16)
        for k in range(N_CORES):
            m = src_core == k
            ss, dd = src_slot[m], dst_g[m]
            kblk = ss // P
            kbat = kblk // B_PP
            kchk = dd // CHUNK
            o = np.lexsort((dd, kblk, kchk, kbat))
            ss, dd, kblk, kbat, kchk = ss[o], dd[o], kblk[o], kbat[o], kchk[o]
            idx_flat = np.zeros(self.total_slots, dtype=np.int64)
            srcrel_flat = np.full(self.total_slots, -1.0, dtype=np.float16)
            # fill run by run
            key = (kbat * self.nchunks + kchk) * self.blocks + kblk
            nkeys = self.nbatches * self.nchunks * self.blocks
            bounds = np.searchsorted(key, np.arange(nkeys + 1))
            for b in range(self.nbatches):
                blo, bhi = b * B_PP, min((b + 1) * B_PP, self.blocks)
                for c in range(self.nchunks):
                    for bb in range(blo, bhi):
                        L = run_len[b, c, bb]
                        if L == 0:
                            continue
                        kk = (b * self.nchunks + c) * self.blocks + bb
                        lo_, hi_ = bounds[kk], bounds[kk + 1]
                        cnt = hi_ - lo_
                        assert cnt <= L
                        base = run_tile_off[b, c, bb] * P
                        idx_flat[base:base + cnt] = dd[lo_:hi_] - c * CHUNK
                        srcrel_flat[base:base + cnt] = (
                            ss[lo_:hi_] - bb * P).astype(np.float16)
                        # pad slots: idx 0 (valid row in chunk), srcrel -1
            assert idx_flat.min() >= 0 and idx_flat.max() < 32768
            w16 = idx_flat.reshape(self.total_slots // 16, 16).T.astype(np.int16)
            self.idx16[k] = np.tile(w16, (8, 1))
            self.srcrel[k] = srcrel_flat.reshape(self.total_tiles, P).T

    def make_tab(self, left, right):
        """Pre-split hi/lo fp16 table [n_pad, CH] of T0 = f * reps."""
        reps = np.concatenate([left, right], axis=0).astype(np.float32)
        t0 = self.f[:, None] * reps
        hi = t0.astype(np.float16)
        lo = ((t0 - hi.astype(np.float32)) * LO_SCALE).astype(np.float16)
        tab = np.zeros((self.n_pad, CH), dtype=np.float16)
        tab[self.gid_of, :D] = hi
        tab[self.gid_of, D:] = lo
        return tab

    def f2_arr(self, k):
        """Per-core [P, blocks] fp32 of f^2/2 at (slot%P, block)."""
        arr = np.zeros(self.n_pad, dtype=np.float32)
        arr[self.gid_of] = self.f * self.f * 0.5
        sl = arr[k * self.n_slice_pad:(k + 1) * self.n_slice_pad]
        return np.ascontiguousarray(sl.reshape(self.blocks, P).T)

    def unpermute(self, acc_slices):
        full = np.concatenate(acc_slices, axis=0)
        return full[self.gid_of]


def build_program(plans):
    nc = bacc.Bacc("TRN2", target_bir_lowering=False, debug=False,
                   num_devices=N_CORES, num_swdge_queues=GQ)

    tabs, idxs, srcs, f2s = {}, {}, {}, {}
    for gp in plans:
        tabs[gp.name] = nc.declare_dram_parameter(
            f"tab_{gp.name}", [gp.n_pad, CH], mybir.dt.float16, isOutput=False)
        idxs[gp.name] = nc.declare_dram_parameter(
            f"idx_{gp.name}", [P, gp.total_slots // 16], mybir.dt.int16,
            isOutput=False)
        srcs[gp.name] = nc.declare_dram_parameter(
            f"srcrel_{gp.name}", [P, gp.total_tiles], mybir.dt.float16,
            isOutput=False)
        f2s[gp.name] = nc.declare_dram_parameter(
            f"f2_{gp.name}", [P, gp.blocks], mybir.dt.float32, isOutput=False)
    out_blocks = sum(gp.blocks for gp in plans)
    reps_own = nc.declare_dram_parameter(
        "reps_own", [P, out_blocks * D], mybir.dt.float32, isOutput=False)
    iota_in = nc.declare_dram_parameter(
        "iota", [P, P], mybir.dt.float16, isOutput=False)
    acc_out = nc.declare_dram_parameter(
        "acc_out", [P, out_blocks * D], mybir.dt.float32, isOutput=True)

    acc1 = nc.dram_tensor("acc1", [P, out_blocks * D], mybir.dt.float32)
    ag_in, ag_out = {}, {}
    for gp in plans:
        ag_in[gp.name] = nc.dram_tensor(
            f"ag_in_{gp.name}", [gp.n_slice_pad, CH], mybir.dt.float16)
        ag_out[gp.name] = nc.dram_tensor(
            f"ag_out_{gp.name}", [gp.n_pad, CH], mybir.dt.float16,
            addr_space="Shared")

    gq_counter = [0]
    NI_T = NI_MAX // P

    with tile.TileContext(nc) as tc:
        with tc.tile_pool(name="const", bufs=1) as constp, \
             tc.tile_pool(name="meta", bufs=3) as metap, \
             tc.tile_pool(name="srp", bufs=2) as srp, \
             tc.tile_pool(name="gpool", bufs=3) as gpool, \
             tc.tile_pool(name="wpool", bufs=2) as wpool, \
             tc.tile_pool(name="stage", bufs=2) as stagep, \
             tc.tile_pool(name="post", bufs=2) as postp, \
             tc.tile_pool(name="psum", bufs=8, space="PSUM") as psump:

            iota_t = constp.tile([P, P], mybir.dt.float16)
            nc.sync.dma_start(out=iota_t[:], in_=iota_in[:, :])
            f2_t = {}
            for gp in plans:
                f2_t[gp.name] = constp.tile([P, gp.blocks], mybir.dt.float32,
                                            name=f"f2t_{gp.name}",
                                            tag=f"f2_{gp.name}")
                nc.sync.dma_start(out=f2_t[gp.name][:], in_=f2s[gp.name][:, :])

            def do_layer(gp, table, acc_prev, acc_next, write_t1):
                for b in range(gp.nbatches):
                    blo = b * B_PP
                    nb = min(B_PP, gp.blocks - blo)
                    bt0 = gp.batch_first_tile[b]
                    btn = gp.batch_tiles[b]
                    stg = stagep.tile([P, B_PP * CH], mybir.dt.float32,
                                      tag="stg")
                    sr = srp.tile([P, gp.max_batch_tiles], mybir.dt.float16,
                                  tag="sr")
                    nc.sync.dma_start(out=sr[:, :btn],
                                      in_=srcs[gp.name][:, bt0:bt0 + btn])
                    for (c, toff, nt, segs) in gp.pieces[b]:
                        ni = nt * P
                        so = toff * P
                        cbase = c * CHUNK
                        csz = min(CHUNK, gp.n_pad - cbase)
                        it = metap.tile([P, NI_MAX // 16], mybir.dt.int16,
                                        tag="idx")
                        nc.sync.dma_start(
                            out=it[:, :ni // 16],
                            in_=idxs[gp.name][:, so // 16:(so + ni) // 16])
                        g = gpool.tile([P, NI_T * CH], mybir.dt.float16,
                                       tag="g")
                        nc.gpsimd.dma_gather(
                            g[:, :nt * CH].rearrange("p (t c) -> p t c", c=CH),
                            table[cbase:cbase + csz, :],
                            it[:, :ni // 16],
                            ni, ni, CH,
                            queue_num=gq_counter[0] % GQ,
                            single_packet=False,
                        )
                        gq_counter[0] += 1
                        w = wpool.tile([P, NI_T * P], mybir.dt.float16, tag="w")
                        bt = toff - bt0
                        nc.vector.tensor_tensor(
                            out=w[:, :nt * P].rearrange("p (t q) -> p t q", q=P),
                            in0=sr[:, bt:bt + nt].to_broadcast([P, nt, P]),
                            in1=iota_t[:, None, :].to_broadcast([P, nt, P]),
                            op=mybir.AluOpType.is_equal)
                        for (bb, st, snt, first) in segs:
                            ps = psump.tile([P, CH], mybir.dt.float32,
                                            padded_shape=[P, 512], tag="ps")
                            for t in range(st, st + snt):
                                nc.tensor.matmul(
                                    out=ps[:],
                                    lhsT=w[:, t * P:(t + 1) * P],
                                    rhs=g[:, t * CH:(t + 1) * CH],
                                    start=(t == st),
                                    stop=(t == st + snt - 1))
                            col = stg[:, bb * CH:(bb + 1) * CH]
                            if first:
                                nc.scalar.mul(col, ps[:], 1.0)
                            else:
                                nc.vector.tensor_tensor(
                                    out=col, in0=col, in1=ps[:],
                                    op=mybir.AluOpType.add)
                    # ---- epilogue for this batch ----
                    psv = stg[:, :nb * CH].rearrange("p (b c) -> p b c", c=CH)
                    s3 = postp.tile([P, B_PP * D], mybir.dt.float32, tag="s3")
                    sv = s3[:, :nb * D].rearrange("p (b d) -> p b d", d=D)
                    # S = hi + lo/2048
                    nc.scalar.mul(sv, psv[:, :, D:CH], 1.0 / LO_SCALE)
                    nc.vector.tensor_tensor(out=sv, in0=sv, in1=psv[:, :, 0:D],
                                            op=mybir.AluOpType.add)
                    sq = postp.tile([P, B_PP * D], mybir.dt.float32, tag="sq")
                    nc.vector.tensor_tensor(
                        out=sq[:, :nb * D], in0=s3[:, :nb * D],
                        in1=s3[:, :nb * D], op=mybir.AluOpType.mult)
                    ssum = postp.tile([P, B_PP], mybir.dt.float32, tag="ssum")
                    nc.vector.tensor_reduce(
                        out=ssum[:, :nb],
                        in_=sq[:, :nb * D].rearrange("p (b d) -> p b d", d=D),
                        axis=mybir.AxisListType.X,
                        op=mybir.AluOpType.add)
                    nc.vector.tensor_scalar_add(
                        out=ssum[:, :nb], in0=ssum[:, :nb], scalar1=EPS_RSQ)
                    nrm = postp.tile([P, B_PP], mybir.dt.float32, tag="nrm")
                    nc.scalar.activation(
                        out=nrm[:, :nb], in_=ssum[:, :nb],
                        func=mybir.ActivationFunctionType.Sqrt)
                    rsq = postp.tile([P, B_PP], mybir.dt.float32, tag="rsq")
                    nc.vector.reciprocal(out=rsq[:, :nb], in_=nrm[:, :nb])
                    prev = postp.tile([P, B_PP * D], mybir.dt.float32, tag="pv")
                    nc.sync.dma_start(
                        out=prev[:, :nb * D],
                        in_=acc_prev[:, blo * D:(blo + nb) * D])
                    normed = postp.tile([P, B_PP * D], mybir.dt.float32,
                                        tag="nd")
                    nc.vector.tensor_tensor(
                        out=normed[:, :nb * D].rearrange(
                            "p (b d) -> p b d", d=D),
                        in0=sv,
                        in1=rsq[:, :nb].to_broadcast([P, nb, D]),
                        op=mybir.AluOpType.mult)
                    accn = postp.tile([P, B_PP * D], mybir.dt.float32, tag="an")
                    nc.vector.tensor_tensor(
                        out=accn[:, :nb * D], in0=prev[:, :nb * D],
                        in1=normed[:, :nb * D], op=mybir.AluOpType.add)
                    nc.sync.dma_start(
                        out=acc_next[:, blo * D:(blo + nb) * D],
                        in_=accn[:, :nb * D])
                    if write_t1:
                        # T1 = (f^2/2) * S, split hi/lo fp16
                        t1f = postp.tile([P, B_PP * D], mybir.dt.float32,
                                         tag="t1f")
                        nc.vector.tensor_tensor(
                            out=t1f[:, :nb * D].rearrange(
                                "p (b d) -> p b d", d=D),
                            in0=sv,
                            in1=f2_t[gp.name][:, blo:blo + nb]
                                .to_broadcast([P, nb, D]),
                            op=mybir.AluOpType.mult)
                        t1c = postp.tile([P, B_PP * CH], mybir.dt.float16,
                                         tag="t1c")
                        t1cv = t1c[:, :nb * CH].rearrange(
                            "p (b c) -> p b c", c=CH)
                        t1fv = t1f[:, :nb * D].rearrange(
                            "p (b d) -> p b d", d=D)
                        nc.scalar.mul(t1cv[:, :, 0:D], t1fv, 1.0)
                        hi32 = postp.tile([P, B_PP * D], mybir.dt.float32,
                                          tag="h32")
                        nc.scalar.mul(
                            hi32[:, :nb * D].rearrange("p (b d) -> p b d", d=D),
                            t1cv[:, :, 0:D], 1.0)
                        # reuse sq as lo32 scratch
                        nc.vector.tensor_tensor(
                            out=sq[:, :nb * D], in0=t1f[:, :nb * D],
                            in1=hi32[:, :nb * D], op=mybir.AluOpType.subtract)
                        nc.scalar.mul(
                            t1cv[:, :, D:CH],
                            sq[:, :nb * D].rearrange("p (b d) -> p b d", d=D),
                            LO_SCALE)
                        nc.sync.dma_start(
                            out=ag_in[gp.name][blo * P:blo * P + nb * P, :]
                                .rearrange("(b p) c -> p b c", p=P),
                            in_=t1cv)

            blk0 = 0
            for gp in plans:
                c0, c1 = blk0 * D, (blk0 + gp.blocks) * D
                do_layer(gp, tabs[gp.name], reps_own[:, c0:c1],
                         acc1[:, c0:c1], write_t1=True)
                blk0 += gp.blocks
                nc.gpsimd.collective_compute(
                    "AllGather",
                    mybir.AluOpType.bypass,
                    ins=[ag_in[gp.name][:, :]],
                    outs=[ag_out[gp.name][:, :]],
                    replica_groups=[list(range(N_CORES))],
                )
            blk0 = 0
            for gp in plans:
                c0, c1 = blk0 * D, (blk0 + gp.blocks) * D
                do_layer(gp, ag_out[gp.name], acc1[:, c0:c1],
                         acc_out[:, c0:c1], write_t1=False)
                blk0 += gp.blocks

    nc.compile()
    return nc


def _make_plans(inputs):
    plans = []
    for name, lk, rk, sk, dk, vk in GRAPHS:
        n = inputs[lk].shape[0] + inputs[rk].shape[0]
        plans.append(GraphPlan(
            name, n, np.asarray(inputs[sk]), np.asarray(inputs[dk])))
    return plans


def _run(inputs, use_dma_gather=True, trace=False):
    users = np.asarray(inputs["users"], dtype=np.float32)
    bundles = np.asarray(inputs["bundles"], dtype=np.float32)
    items = np.asarray(inputs["items"], dtype=np.float32)
    halves = {"ui": (users, items), "ub": (users, bundles),
              "bi": (bundles, items)}

    plans = _make_plans(inputs)
    nc = build_program(plans)

    iota = np.tile(np.arange(P, dtype=np.float16)[None, :], (P, 1))
    in_maps = []
    for k in range(N_CORES):
        m = {"iota": iota}
        reps_parts = []
        for gp in plans:
            left, right = halves[gp.name]
            m[f"tab_{gp.name}"] = gp.make_tab(left, right)
            m[f"idx_{gp.name}"] = gp.idx16[k]
            m[f"srcrel_{gp.name}"] = gp.srcrel[k]
            m[f"f2_{gp.name}"] = gp.f2_arr(k)
            reps = np.concatenate([left, right], axis=0)
            tabf = np.zeros((gp.n_pad, D), dtype=np.float32)
            tabf[gp.gid_of] = reps
            reps_parts.append(
                tabf[k * gp.n_slice_pad:(k + 1) * gp.n_slice_pad])
        pm = [r.reshape(-1, P, D).transpose(1, 0, 2).reshape(P, -1)
              for r in reps_parts]
        m["reps_own"] = np.ascontiguousarray(np.concatenate(pm, axis=1))
        in_maps.append(m)

    res = run_bass_kernel_spmd(nc, in_maps, list(range(N_CORES)), trace=trace)

    acc = {}
    blk0 = 0
    for gp in plans:
        slices = []
        for k in range(N_CORES):
            a = res.results[k]["acc_out"][:, blk0 * D:(blk0 + gp.blocks) * D]
            a = a.reshape(P, gp.blocks, D).transpose(1, 0, 2).reshape(-1, D)
            slices.append(a)
        acc[gp.name] = gp.unpermute(np.stack(slices))
        blk0 += gp.blocks

    NU, NB = users.shape[0], bundles.shape[0]
    il_u, il_i = acc["ui"][:NU], acc["ui"][NU:]
    bl_u, bl_b = acc["ub"][:NU], acc["ub"][NU:]
    bs_b, bs_i = acc["bi"][:NB], acc["bi"][NB:]
    out = np.concatenate([il_u, bl_u, bl_b, bs_b, il_i, bs_i], axis=0)
    return out, res


def kernel(**inputs) -> np.ndarray:
    out, _ = _run(inputs, trace=False)
    return out


# revision 13
# speedup vs baseline: 1.2861x; 1.2861x over previous
"""Bass/Trainium2 kernel for nn_BMGAE (LightGCN-style 2-layer propagation on
three bipartite graphs), sharded across 8 NeuronCores.

v2 strategy (vs baseline):
  - val factorization: val_e = f(src)*f(dst) with f = 1/(sqrt(deg)+eps), so the
    gathered table is premultiplied T_l = f * cur_l and the per-edge val
    multiply disappears. l2norm is scale-invariant, so the 1/(l+2) and f(s)
    factors vanish from the normalize path; only the next-layer table build
    needs f^2/2 (a per-node epilogue op).
  - tables stored as pre-split hi/lo fp16 pairs [hi64|lo64] (256B/row, same
    bytes as fp32): gathered tiles feed the fp16 matmul directly with no
    per-edge casts. Layer-0 table split on host; layer-1 split per-node in the
    epilogue before the AllGather.
  - big gather pieces (up to NI_MAX indices per DMAGather) sorted by
    (batch-of-16-blocks, chunk, block, dst) amortize the ~1us fixed SWDGE cost;
    PSUM holds 16 block accumulators (4 banks) double-buffered.

kernel(**inputs) takes the FULL unsharded inputs and returns the FULL output.
"""
import numpy as np

import concourse.tile as tile
from concourse import bass, bacc, mybir
from concourse.bass_utils import run_bass_kernel_spmd

P = 128
N_CORES = 8
D = 64
CH = 2 * D            # gathered row: [hi(64) | lo(64)] fp16 = 256B
LO_SCALE = 2048.0
EPS_DEG = 1e-8
EPS_RSQ = 1e-24
B_PP = 16             # blocks per SBUF staging batch
GQ = 4                # SWDGE queues
CHUNK = 32768         # int16 dma_gather index range per table chunk
NI_MAX = 2048         # max indices per dma_gather instruction

GRAPHS = [
    ("ui", "users", "items", "ui_src", "ui_dst", "ui_val"),
    ("ub", "users", "bundles", "ub_src", "ub_dst", "ub_val"),
    ("bi", "bundles", "items", "bi_src", "bi_dst", "bi_val"),
]


def _ceil(a, b):
    return -(-a // b)


class GraphPlan:
    """Host-side plan for one graph: permutation, piece/tile schedule."""

    def __init__(self, name, n, src, dst):
        self.name = name
        self.n = n
        deg = np.bincount(src, minlength=n)
        # f = 1/(sqrt(deg)+eps), matching reference fp32 ops
        self.f = (1.0 / (np.sqrt(deg.astype(np.float32)) + np.float32(EPS_DEG))
                  ).astype(np.float32)
        order = np.argsort(-deg, kind="stable")
        rank = np.empty(n, dtype=np.int64)
        rank[order] = np.arange(n)
        self.core_of = (rank % N_CORES).astype(np.int64)
        j = rank // N_CORES
        self.n_slice_pad = _ceil(_ceil(n, N_CORES), P) * P
        self.blocks = self.n_slice_pad // P
        self.slot_of = (j % self.blocks) * P + j // self.blocks
        self.n_pad = self.n_slice_pad * N_CORES
        self.gid_of = self.core_of * self.n_slice_pad + self.slot_of
        self.nchunks = _ceil(self.n_pad, CHUNK)
        self.nbatches = _ceil(self.blocks, B_PP)

        dst_g = self.gid_of[dst]
        src_core = self.core_of[src]
        src_slot = self.slot_of[src]

        # counts per (core, batch, chunk, block) -> shared run lengths (SPMD)
        blk = src_slot // P
        bat = blk // B_PP
        chk = dst_g // CHUNK
        counts = np.zeros((N_CORES, self.nbatches, self.nchunks, self.blocks),
                          dtype=np.int64)
        np.add.at(counts, (src_core, bat, chk, blk), 1)
        run_len = _ceil(np.maximum(counts.max(axis=0), 0), P) * P  # [bat,chk,blk]
        # zero-length runs stay zero
        run_len[counts.max(axis=0) == 0] = 0
        self.run_len = run_len

        # run order: (bat, chk, blk) with blk restricted to its batch
        # build tile schedule
        self.total_slots = int(run_len.sum())
        self.total_tiles = self.total_slots // P
        run_tile_off = np.zeros_like(run_len)
        t = 0
        self.batch_first_tile = []
        self.batch_tiles = []
        # pieces: list per batch of (chunk, tile_off, ntiles, segments)
        # segment: (block_in_batch, tile_off_in_piece, ntiles, is_first_spill)
        self.pieces = [[] for _ in range(self.nbatches)]
        tile_block = []
        max_ni_t = NI_MAX // P
        for b in range(self.nbatches):
            bt0 = t
            blo, bhi = b * B_PP, min((b + 1) * B_PP, self.blocks)
            seen = set()  # blocks already spilled once in this batch

            def close_piece(c, p_t0, p_nt, p_segs):
                self.pieces[b].append((c, p_t0, p_nt, p_segs))

            for c in range(self.nchunks):
                p_t0 = t
                p_nt = 0
                p_segs = []
                for bb in range(blo, bhi):
                    L = run_len[b, c, bb]
                    if L == 0:
                        continue
                    run_tile_off[b, c, bb] = t
                    ltiles = L // P
                    done = 0
                    while done < ltiles:
                        take = min(ltiles - done, max_ni_t - p_nt)
                        p_segs.append((bb - blo, p_nt, take, bb not in seen))
                        seen.add(bb)
                        tile_block.extend([bb - blo] * take)
                        t += take
                        p_nt += take
                        done += take
                        if p_nt == max_ni_t:
                            close_piece(c, p_t0, p_nt, p_segs)
                            p_t0 = t
                            p_nt = 0
                            p_segs = []
                if p_nt > 0:
                    close_piece(c, p_t0, p_nt, p_segs)
            # every block in the batch must have been spilled at least once
            assert len(seen) == bhi - blo, (self.name, b, seen)
            self.batch_first_tile.append(bt0)
            self.batch_tiles.append(t - bt0)
        assert t == self.total_tiles
        self.run_tile_off = run_tile_off
        self.tile_block = np.array(tile_block, dtype=np.int64)
        self.max_batch_tiles = max(self.batch_tiles)

        # ---- per-core slot arrays ----
        self.idx16 = np.zeros((N_CORES, P, self.total_slots // 16), np.int16)
        self.srcrel = np.full((N_CORES, P, self.total_tiles), -1.0, np.float16)
        for k in range(N_CORES):
            m = src_core == k
            ss, dd = src_slot[m], dst_g[m]
            kblk = ss // P
            kbat = kblk // B_PP
            kchk = dd // CHUNK
            o = np.lexsort((dd, kblk, kchk, kbat))
            ss, dd, kblk, kbat, kchk = ss[o], dd[o], kblk[o], kbat[o], kchk[o]
            idx_flat = np.zeros(self.total_slots, dtype=np.int64)
            srcrel_flat = np.full(self.total_slots, -1.0, dtype=np.float16)
            # fill run by run
            key = (kbat * self.nchunks + kchk) * self.blocks + kblk
            nkeys = self.nbatches * self.nchunks * self.blocks
            bounds = np.searchsorted(key, np.arange(nkeys + 1))
            for b in range(self.nbatches):
                blo, bhi = b * B_PP, min((b + 1) * B_PP, self.blocks)
                for c in range(self.nchunks):
                    for bb in range(blo, bhi):
                        L = run_len[b, c, bb]
                        if L == 0:
                            continue
                        kk = (b * self.nchunks + c) * self.blocks + bb
                        lo_, hi_ = bounds[kk], bounds[kk + 1]
                        cnt = hi_ - lo_
                        assert cnt <= L
                        base = run_tile_off[b, c, bb] * P
                        idx_flat[base:base + cnt] = dd[lo_:hi_] - c * CHUNK
                        srcrel_flat[base:base + cnt] = (
                            ss[lo_:hi_] - bb * P).astype(np.float16)
                        # pad slots: idx 0 (valid row in chunk), srcrel -1
            assert idx_flat.min() >= 0 and idx_flat.max() < 32768
            w16 = idx_flat.reshape(self.total_slots // 16, 16).T.astype(np.int16)
            self.idx16[k] = np.tile(w16, (8, 1))
            self.srcrel[k] = srcrel_flat.reshape(self.total_tiles, P).T

    def make_tab(self, left, right):
        """Pre-split hi/lo fp16 table [n_pad, CH] of T0 = f * reps."""
        reps = np.concatenate([left, right], axis=0).astype(np.float32)
        t0 = self.f[:, None] * reps
        hi = t0.astype(np.float16)
        lo = ((t0 - hi.astype(np.float32)) * LO_SCALE).astype(np.float16)
        tab = np.zeros((self.n_pad, CH), dtype=np.float16)
        tab[self.gid_of, :D] = hi
        tab[self.gid_of, D:] = lo
        return tab

    def f2_arr(self, k):
        """Per-core [P, blocks] fp32 of f^2/2 at (slot%P, block)."""
        arr = np.zeros(self.n_pad, dtype=np.float32)
        arr[self.gid_of] = self.f * self.f * 0.5
        sl = arr[k * self.n_slice_pad:(k + 1) * self.n_slice_pad]
        return np.ascontiguousarray(sl.reshape(self.blocks, P).T)

    def unpermute(self, acc_slices):
        full = np.concatenate(acc_slices, axis=0)
        return full[self.gid_of]


def build_program(plans):
    nc = bacc.Bacc("TRN2", target_bir_lowering=False, debug=False,
                   num_devices=N_CORES, num_swdge_queues=GQ)

    tabs, idxs, srcs, f2s = {}, {}, {}, {}
    for gp in plans:
        tabs[gp.name] = nc.declare_dram_parameter(
            f"tab_{gp.name}", [gp.n_pad, CH], mybir.dt.float16, isOutput=False)
        idxs[gp.name] = nc.declare_dram_parameter(
            f"idx_{gp.name}", [P, gp.total_slots // 16], mybir.dt.int16,
            isOutput=False)
        srcs[gp.name] = nc.declare_dram_parameter(
            f"srcrel_{gp.name}", [P, gp.total_tiles], mybir.dt.float16,
            isOutput=False)
        f2s[gp.name] = nc.declare_dram_parameter(
            f"f2_{gp.name}", [P, gp.blocks], mybir.dt.float32, isOutput=False)
    out_blocks = sum(gp.blocks for gp in plans)
    reps_own = nc.declare_dram_parameter(
        "reps_own", [P, out_blocks * D], mybir.dt.float32, isOutput=False)
    iota_in = nc.declare_dram_parameter(
        "iota", [P, P], mybir.dt.float16, isOutput=False)
    acc_out = nc.declare_dram_parameter(
        "acc_out", [P, out_blocks * D], mybir.dt.float32, isOutput=True)

    acc1 = nc.dram_tensor("acc1", [P, out_blocks * D], mybir.dt.float32)
    ag_in, ag_out = {}, {}
    for gp in plans:
        ag_in[gp.name] = nc.dram_tensor(
            f"ag_in_{gp.name}", [gp.n_slice_pad, CH], mybir.dt.float16)
        ag_out[gp.name] = nc.dram_tensor(
            f"ag_out_{gp.name}", [gp.n_pad, CH], mybir.dt.float16,
            addr_space="Shared")

    gq_counter = [0]
    NI_T = NI_MAX // P

    with tile.TileContext(nc) as tc:
        with tc.tile_pool(name="const", bufs=1) as constp, \
             tc.tile_pool(name="meta", bufs=5) as metap, \
             tc.tile_pool(name="srp", bufs=3) as srp, \
             tc.tile_pool(name="gpool", bufs=5) as gpool, \
             tc.tile_pool(name="wpool", bufs=4) as wpool, \
             tc.tile_pool(name="stage", bufs=3) as stagep, \
             tc.tile_pool(name="post", bufs=2) as postp, \
             tc.tile_pool(name="psum", bufs=8, space="PSUM") as psump:

            iota_t = constp.tile([P, P], mybir.dt.float16)
            nc.sync.dma_start(out=iota_t[:], in_=iota_in[:, :])
            eps_t = constp.tile([P, B_PP], mybir.dt.float32)
            nc.vector.memset(eps_t[:], EPS_RSQ)
            f2_t = {}
            for gp in plans:
                f2_t[gp.name] = constp.tile([P, gp.blocks], mybir.dt.float32,
                                            name=f"f2t_{gp.name}",
                                            tag=f"f2_{gp.name}")
                nc.sync.dma_start(out=f2_t[gp.name][:], in_=f2s[gp.name][:, :])

            def do_layer(gp, table, acc_prev, acc_next, write_t1):
                for b in range(gp.nbatches):
                    blo = b * B_PP
                    nb = min(B_PP, gp.blocks - blo)
                    bt0 = gp.batch_first_tile[b]
                    btn = gp.batch_tiles[b]
                    stg = stagep.tile([P, B_PP * CH], mybir.dt.float32,
                                      tag="stg")
                    sr = srp.tile([P, gp.max_batch_tiles], mybir.dt.float16,
                                  tag="sr")
                    nc.sync.dma_start(out=sr[:, :btn],
                                      in_=srcs[gp.name][:, bt0:bt0 + btn])
                    for (c, toff, nt, segs) in gp.pieces[b]:
                        ni = nt * P
                        so = toff * P
                        cbase = c * CHUNK
                        csz = min(CHUNK, gp.n_pad - cbase)
                        it = metap.tile([P, NI_MAX // 16], mybir.dt.int16,
                                        tag="idx")
                        nc.sync.dma_start(
                            out=it[:, :ni // 16],
                            in_=idxs[gp.name][:, so // 16:(so + ni) // 16])
                        g = gpool.tile([P, NI_T * CH], mybir.dt.float16,
                                       tag="g")
                        nc.gpsimd.dma_gather(
                            g[:, :nt * CH].rearrange("p (t c) -> p t c", c=CH),
                            table[cbase:cbase + csz, :],
                            it[:, :ni // 16],
                            ni, ni, CH,
                            queue_num=gq_counter[0] % GQ,
                            single_packet=False,
                        )
                        gq_counter[0] += 1
                        w = wpool.tile([P, NI_T * P], mybir.dt.float16, tag="w")
                        bt = toff - bt0
                        nc.vector.tensor_tensor(
                            out=w[:, :nt * P].rearrange("p (t q) -> p t q", q=P),
                            in0=sr[:, bt:bt + nt].to_broadcast([P, nt, P]),
                            in1=iota_t[:, None, :].to_broadcast([P, nt, P]),
                            op=mybir.AluOpType.is_equal)
                        for (bb, st, snt, first) in segs:
                            ps = psump.tile([P, CH], mybir.dt.float32,
                                            padded_shape=[P, 512], tag="ps")
                            for t in range(st, st + snt):
                                nc.tensor.matmul(
                                    out=ps[:],
                                    lhsT=w[:, t * P:(t + 1) * P],
                                    rhs=g[:, t * CH:(t + 1) * CH],
                                    start=(t == st),
                                    stop=(t == st + snt - 1))
                            col = stg[:, bb * CH:(bb + 1) * CH]
                            if first:
                                nc.scalar.mul(col, ps[:], 1.0)
                            else:
                                nc.vector.tensor_tensor(
                                    out=col, in0=col, in1=ps[:],
                                    op=mybir.AluOpType.add)
                    # ---- epilogue for this batch ----
                    psv = stg[:, :nb * CH].rearrange("p (b c) -> p b c", c=CH)
                    s3 = postp.tile([P, B_PP * D], mybir.dt.float32, tag="s3")
                    sv = s3[:, :nb * D].rearrange("p (b d) -> p b d", d=D)
                    # S = hi + lo/2048
                    nc.scalar.mul(sv, psv[:, :, D:CH], 1.0 / LO_SCALE)
                    nc.vector.tensor_tensor(out=sv, in0=sv, in1=psv[:, :, 0:D],
                                            op=mybir.AluOpType.add)
                    sq = postp.tile([P, B_PP * D], mybir.dt.float32, tag="sq")
                    nc.vector.tensor_tensor(
                        out=sq[:, :nb * D], in0=s3[:, :nb * D],
                        in1=s3[:, :nb * D], op=mybir.AluOpType.mult)
                    ssum = postp.tile([P, B_PP], mybir.dt.float32, tag="ssum")
                    nc.vector.tensor_reduce(
                        out=ssum[:, :nb],
                        in_=sq[:, :nb * D].rearrange("p (b d) -> p b d", d=D),
                        axis=mybir.AxisListType.X,
                        op=mybir.AluOpType.add)
                    nc.vector.tensor_tensor(
                        out=ssum[:, :nb], in0=ssum[:, :nb],
                        in1=eps_t[:, :nb], op=mybir.AluOpType.add)
                    nrm = postp.tile([P, B_PP], mybir.dt.float32, tag="nrm")
                    nc.scalar.activation(
                        out=nrm[:, :nb], in_=ssum[:, :nb],
                        func=mybir.ActivationFunctionType.Sqrt)
                    rsq = postp.tile([P, B_PP], mybir.dt.float32, tag="rsq")
                    nc.vector.reciprocal(out=rsq[:, :nb], in_=nrm[:, :nb])
                    prev = postp.tile([P, B_PP * D], mybir.dt.float32, tag="pv")
                    nc.sync.dma_start(
                        out=prev[:, :nb * D],
                        in_=acc_prev[:, blo * D:(blo + nb) * D])
                    normed = postp.tile([P, B_PP * D], mybir.dt.float32,
                                        tag="nd")
                    nc.vector.tensor_tensor(
                        out=normed[:, :nb * D].rearrange(
                            "p (b d) -> p b d", d=D),
                        in0=sv,
                        in1=rsq[:, :nb].to_broadcast([P, nb, D]),
                        op=mybir.AluOpType.mult)
                    accn = postp.tile([P, B_PP * D], mybir.dt.float32, tag="an")
                    nc.vector.tensor_tensor(
                        out=accn[:, :nb * D], in0=prev[:, :nb * D],
                        in1=normed[:, :nb * D], op=mybir.AluOpType.add)
                    nc.sync.dma_start(
                        out=acc_next[:, blo * D:(blo + nb) * D],
                        in_=accn[:, :nb * D])
                    if write_t1:
                        # T1 = (f^2/2) * S, split hi/lo fp16
                        t1f = postp.tile([P, B_PP * D], mybir.dt.float32,
                                         tag="t1f")
                        nc.vector.tensor_tensor(
                            out=t1f[:, :nb * D].rearrange(
                                "p (b d) -> p b d", d=D),
                            in0=sv,
                            in1=f2_t[gp.name][:, blo:blo + nb]
                                .to_broadcast([P, nb, D]),
                            op=mybir.AluOpType.mult)
                        t1c = postp.tile([P, B_PP * CH], mybir.dt.float16,
                                         tag="t1c")
                        t1cv = t1c[:, :nb * CH].rearrange(
                            "p (b c) -> p b c", c=CH)
                        t1fv = t1f[:, :nb * D].rearrange(
                            "p (b d) -> p b d", d=D)
                        nc.scalar.mul(t1cv[:, :, 0:D], t1fv, 1.0)
                        hi32 = postp.tile([P, B_PP * D], mybir.dt.float32,
                                          tag="h32")
                        nc.scalar.mul(
                            hi32[:, :nb * D].rearrange("p (b d) -> p b d", d=D),
                            t1cv[:, :, 0:D], 1.0)
                        # reuse sq as lo32 scratch
                        nc.vector.tensor_tensor(
                            out=sq[:, :nb * D], in0=t1f[:, :nb * D],
                            in1=hi32[:, :nb * D], op=mybir.AluOpType.subtract)
                        nc.scalar.mul(
                            t1cv[:, :, D:CH],
                            sq[:, :nb * D].rearrange("p (b d) -> p b d", d=D),
                            LO_SCALE)
                        nc.sync.dma_start(
                            out=ag_in[gp.name][blo * P:blo * P + nb * P, :]
                                .rearrange("(b p) c -> p b c", p=P),
                            in_=t1cv)

            blk0 = 0
            for gp in plans:
                c0, c1 = blk0 * D, (blk0 + gp.blocks) * D
                do_layer(gp, tabs[gp.name], reps_own[:, c0:c1],
                         acc1[:, c0:c1], write_t1=True)
                blk0 += gp.blocks
                nc.gpsimd.collective_compute(
                    "AllGather",
                    mybir.AluOpType.bypass,
                    ins=[ag_in[gp.name][:, :]],
                    outs=[ag_out[gp.name][:, :]],
                    replica_groups=[list(range(N_CORES))],
                )
            blk0 = 0
            for gp in plans:
                c0, c1 = blk0 * D, (blk0 + gp.blocks) * D
                do_layer(gp, ag_out[gp.name], acc1[:, c0:c1],
                         acc_out[:, c0:c1], write_t1=False)
                blk0 += gp.blocks

    nc.compile()
    return nc


def _make_plans(inputs):
    plans = []
    for name, lk, rk, sk, dk, vk in GRAPHS:
        n = inputs[lk].shape[0] + inputs[rk].shape[0]
        plans.append(GraphPlan(
            name, n, np.asarray(inputs[sk]), np.asarray(inputs[dk])))
    return plans


def _run(inputs, use_dma_gather=True, trace=False):
    users = np.asarray(inputs["users"], dtype=np.float32)
    bundles = np.asarray(inputs["bundles"], dtype=np.float32)
    items = np.asarray(inputs["items"], dtype=np.float32)
    halves = {"ui": (users, items), "ub": (users, bundles),
              "bi": (bundles, items)}

    plans = _make_plans(inputs)
    nc = build_program(plans)

    iota = np.tile(np.arange(P, dtype=np.float16)[None, :], (P, 1))
    in_maps = []
    for k in range(N_CORES):
        m = {"iota": iota}
        reps_parts = []
        for gp in plans:
            left, right = halves[gp.name]
            m[f"tab_{gp.name}"] = gp.make_tab(left, right)
            m[f"idx_{gp.name}"] = gp.idx16[k]
            m[f"srcrel_{gp.name}"] = gp.srcrel[k]
            m[f"f2_{gp.name}"] = gp.f2_arr(k)
            reps = np.concatenate([left, right], axis=0)
            tabf = np.zeros((gp.n_pad, D), dtype=np.float32)
            tabf[gp.gid_of] = reps
            reps_parts.append(
                tabf[k * gp.n_slice_pad:(k + 1) * gp.n_slice_pad])
        pm = [r.reshape(-1, P, D).transpose(1, 0, 2).reshape(P, -1)
              for r in reps_parts]
        m["reps_own"] = np.ascontiguousarray(np.concatenate(pm, axis=1))
        in_maps.append(m)

    res = run_bass_kernel_spmd(nc, in_maps, list(range(N_CORES)), trace=trace)

    acc = {}
    blk0 = 0
    for gp in plans:
        slices = []
        for k in range(N_CORES):
            a = res.results[k]["acc_out"][:, blk0 * D:(blk0 + gp.blocks) * D]
            a = a.reshape(P, gp.blocks, D).transpose(1, 0, 2).reshape(-1, D)
            slices.append(a)
        acc[gp.name] = gp.unpermute(np.stack(slices))
        blk0 += gp.blocks

    NU, NB = users.shape[0], bundles.shape[0]
    il_u, il_i = acc["ui"][:NU], acc["ui"][NU:]
    bl_u, bl_b = acc["ub"][:NU], acc["ub"][NU:]
    bs_b, bs_i = acc["bi"][:NB], acc["bi"][NB:]
    out = np.concatenate([il_u, bl_u, bl_b, bs_b, il_i, bs_i], axis=0)
    return out, res


def kernel(**inputs) -> np.ndarray:
    out, _ = _run(inputs, trace=False)
    return out
